# revision 36
# baseline (speedup 1.0000x reference)
"""Trainium2 8-core kernel for nn_Attention_88948772700322.

Reference computes (N=1024, B=4, C=1024, H=16, hd=64):
    qkv = x @ w_qkv.T                      [N,B,3C]
    q,k,v per (b,h); attn = softmax(q k^T / 8) v
    out = (attn.transpose(2,1,0,3)).reshape(N,B,C) @ w_proj.T + b_proj
The reshape interleaves H and B: proj-input channel c of output-batch bn is
attention head h = 4*bn + c//256, original batch b2 = (c%256)//64, dim d = c%64.

Sharding: tensor-parallel over heads — core i owns heads {2i, 2i+1}, all
batches/tokens (6.44 GFLOP/core, perfectly balanced).  Each core computes a
partial projection over its 512 proj-input channels for output batch bn=i//2;
host sums core pairs (the "all-reduce after proj" realized in unshard).

Host-side prep absorbs every layout nuisance:
  - xT [C, B*N] bf16, tokens batch-major  -> qkv needs no on-chip transpose
  - w_qk [C, 256] (cols q_h0,q_h1,k_h0,k_h1), q pre-scaled by 1/8
  - w_v  [C, 128] (cols v_h0,v_h1)
  - w_p  [512, 1024] = w_proj columns permuted to (b2, h_local, d) row order
On-chip per core: qk^T via PE (d-major), v via PE (token-major), scores
computed transposed (keys on partitions), softmax without max-subtraction
(scores are O(1) by construction), denominator via ones-column in V,
normalization by DMA-partition-broadcast reciprocal, partial proj n-major.

Schedule notes (v2):
  - input DMAs issued chunk-interleaved (wqk[kc] with xc[0][kc]) so the first
    qk matmuls start ~1us in; wp loads last.
  - a few dummy matmuls + one dummy exp at t0 warm the PE HAM clock-gate and
    preload the ACT exp table during the initial DMA window.
  - all PSUM pools stay open for the whole kernel (no mid-kernel pool-close
    DRAIN); proj accumulators reuse the qkv psum ring.
  - the last attention block (b3,qt1) is split in token halves and the eight
    proj waves are emitted as soon as their token range has been normalized,
    so the tile scheduler can interleave proj matmuls into the exp-bound
    endgame; output is written bf16 (host sums partials in f32).
"""

import numpy as np
import ml_dtypes

import concourse.bass as bass
import concourse.mybir as mybir
from concourse import bacc
from concourse.tile import TileContext
from concourse.bass_utils import run_bass_kernel_spmd


N, B, C, H, HD = 1024, 4, 1024, 16, 64
NT = B * N          # 4096 tokens
NCORES = 8
BF = mybir.dt.bfloat16
F32 = mybir.dt.float32
bf16 = ml_dtypes.bfloat16

_NC_CACHE = {}


import os
V2_WARMUP = os.environ.get("V2_WARMUP", "1") == "1"
V2_RB3D = os.environ.get("V2_RB3D", "1") == "1"
V2_SPLIT = os.environ.get("V2_SPLIT", "1") == "1"


def build_nc():
    nc = bacc.Bacc()
    xT_e = nc.declare_dram_parameter("xT", [C, NT], BF, isOutput=False)
    wqk_e = nc.declare_dram_parameter("w_qk", [C, 256], BF, isOutput=False)
    wv_e = nc.declare_dram_parameter("w_v", [C, 128], BF, isOutput=False)
    wp_e = nc.declare_dram_parameter("w_p", [512, C], BF, isOutput=False)
    out_e = nc.declare_dram_parameter("out", [N, C], BF, isOutput=True)

    xT_ap = xT_e[:].rearrange("(co p) t -> p co t", p=128)    # [128, 8, 4096]
    wqk_ap = wqk_e[:].rearrange("(co p) m -> p co m", p=128)  # [128, 8, 256]
    wv_ap = wv_e[:].rearrange("(co p) m -> p co m", p=128)    # [128, 8, 128]
    wp_ap = wp_e[:].rearrange("(b2 p) d -> p b2 d", p=128)    # [128, 4, 1024]

    from contextlib import ExitStack
    with TileContext(nc) as tc:
        with ExitStack() as stk:
            cpool = stk.enter_context(tc.tile_pool(name="const", bufs=1))
            epool = stk.enter_context(tc.tile_pool(name="exp", bufs=6))
            spool = stk.enter_context(tc.tile_pool(name="small", bufs=6))
            opool = stk.enter_context(tc.tile_pool(name="outcp", bufs=8))
            dpool = stk.enter_context(
                tc.tile_pool(name="dram", bufs=4, space="DRAM"))
            ps_qk = stk.enter_context(
                tc.tile_pool(name="ps_qk", bufs=2, space="PSUM"))
            ps_sT = stk.enter_context(
                tc.tile_pool(name="ps_sT", bufs=2, space="PSUM"))
            ps_av = stk.enter_context(
                tc.tile_pool(name="ps_av", bufs=2, space="PSUM"))
            # ---- persistent SBUF tensors -------------------------------
            xc = [[cpool.tile([128, N], BF, name=f"xc_{b}_{kc}")
                   for kc in range(8)] for b in range(B)]
            wqk = cpool.tile([128, 8, 256], BF)
            wv = cpool.tile([128, 8, 128], BF)
            wp = cpool.tile([128, 4, C], BF)
            q_sb = cpool.tile([128, NT], BF)       # [ (h0|h1) d, token ]
            k_sb = cpool.tile([128, NT], BF)
            # v token-major with ones cols: [t_in, t_out, (h0 d64, 1, h1 d64, 1)]
            # cols: 0:64 h0-dims, 64 ones, 65:129 h1-dims, 129 ones
            v_sb = cpool.tile([128, 32, 130], BF)
            projin = cpool.tile([128, B, N], BF)   # [(hl,d), b2, n]

            # ---- warmup: dummy matmuls on the first wqk chunk trip the
            # HAM clock-gate to full speed during the input-DMA window and
            # preload the ACT exp table; they gate only on the very first
            # (64KB) weight DMA. --------------------------------------------
            # (lives in the sT ring: the qk ring is needed immediately by
            # the first qkv block, while the first sT tile isn't touched
            # until attention starts ~25us in)
            wps = None
            if V2_WARMUP:
                wps = ps_sT.tile([128, 2, 512], F32, tag="sT", name="wps")
                for i in range(40):
                    nc.tensor.matmul(wps[:, 0, 0:128], wqk[:, 0, 0:128],
                                     wqk[:, 0, 0:128], start=True, stop=True)
                ewarm = spool.tile([128, 128], BF, tag="ewarm", name="ewarm")
                nc.scalar.activation(ewarm[:], wps[:, 0, 0:128],
                                     mybir.ActivationFunctionType.Exp)

            # ---- input DMAs (sync queue, in consumption order) ---------
            # critical path (batch 0 + weights) on the sync queue; batches
            # 1-3 + wp stream in parallel on the gpsimd queue (norm DMAs
            # live on sync, which is idle again by the time they start)
            for kc in range(8):
                nc.sync.dma_start(out=wqk[:, kc, :], in_=wqk_ap[:, kc, :])
                nc.sync.dma_start(out=xc[0][kc][:], in_=xT_ap[:, kc, 0:N])
            nc.sync.dma_start(out=wv[:], in_=wv_ap)
            for b in range(1, B):
                for kc in range(8):
                    nc.gpsimd.dma_start(out=xc[b][kc][:],
                                        in_=xT_ap[:, kc, b * N:(b + 1) * N])
            nc.gpsimd.dma_start(out=wp[:], in_=wp_ap)

            nc.vector.memset(v_sb[:, :, 64:65], 1.0)
            nc.vector.memset(v_sb[:, :, 129:130], 1.0)

            def qkv_qk(b):
                for tc_i in (2 * b, 2 * b + 1):
                    qps = ps_qk.tile([128, 512], F32, tag="qk",
                                     name=f"qps_{b}_{tc_i}")
                    kps = ps_qk.tile([128, 512], F32, tag="qk",
                                     name=f"kps_{b}_{tc_i}")
                    for kc in range(8):
                        j = tc_i - 2 * b
                        nc.tensor.matmul(qps[:], wqk[:, kc, 0:128],
                                         xc[b][kc][:, j * 512:(j + 1) * 512],
                                         start=(kc == 0), stop=(kc == 7))
                        nc.tensor.matmul(kps[:], wqk[:, kc, 128:256],
                                         xc[b][kc][:, j * 512:(j + 1) * 512],
                                         start=(kc == 0), stop=(kc == 7))
                    nc.vector.tensor_copy(
                        out=q_sb[:, tc_i * 512:(tc_i + 1) * 512], in_=qps[:])
                    nc.vector.tensor_copy(
                        out=k_sb[:, tc_i * 512:(tc_i + 1) * 512], in_=kps[:])

            def qkv_v(b):
                for tt in range(8 * b, 8 * b + 8):
                    vps = ps_qk.tile([128, 128], F32, tag="qk", name=f"vps_{tt}")
                    for kc in range(8):
                        nc.tensor.matmul(vps[:],
                                         xc[b][kc][:, (tt - 8 * b) * 128:
                                                    (tt - 8 * b + 1) * 128],
                                         wv[:, kc, :],
                                         start=(kc == 0), stop=(kc == 7))
                    nc.vector.tensor_copy(out=v_sb[:, tt, 0:64],
                                          in_=vps[:, 0:64])
                    nc.vector.tensor_copy(out=v_sb[:, tt, 65:129],
                                          in_=vps[:, 64:128])

            def attn_block(b, qlo, qw):
                """Attention for q-tokens [qlo, qlo+qw) of batch b."""
                q_sl = slice(b * N + qlo, b * N + qlo + qw)
                av0 = ps_av.tile([65, 512], F32, tag="av",
                                 name=f"av0_{b}_{qlo}")
                av1 = ps_av.tile([65, 512], F32, tag="av",
                                 name=f"av1_{b}_{qlo}")
                avs = [av0, av1]
                for kc in range(8):
                    k_sl = slice(b * N + kc * 128, b * N + (kc + 1) * 128)
                    # [128, 2, 512]: each head's half in its own PSUM bank,
                    # even for qw<512 (two concurrent tile_position matmuls
                    # must not write the same bank)
                    sT = ps_sT.tile([128, 2, 512], F32, tag="sT",
                                    name=f"sT_{b}_{qlo}_{kc}")
                    for hl in range(2):
                        nc.tensor.matmul(
                            sT[:, hl, 0:qw],
                            k_sb[hl * 64:(hl + 1) * 64, k_sl],
                            q_sb[hl * 64:(hl + 1) * 64, q_sl],
                            start=True, stop=True,
                            tile_position=(hl * 64, 0))
                    e = epool.tile([128, 2, 512], BF, tag="e",
                                   name=f"e_{b}_{qlo}_{kc}")
                    nc.scalar.activation(
                        e[:, :, 0:qw], sT[:, :, 0:qw],
                        mybir.ActivationFunctionType.Exp)
                    for hl in range(2):
                        nc.tensor.matmul(
                            avs[hl][:, 0:qw],
                            v_sb[:, 8 * b + kc, hl * 65:(hl + 1) * 65],
                            e[:, hl, 0:qw],
                            start=(kc == 0), stop=(kc == 7))
                return avs

            def norm_block(b, qlo, qw, avs):
                # evacuate av psum -> sbuf (frees psum; DMA can then read den)
                qv = qw // 8
                av_sb = spool.tile([65, 2, 512], F32, tag="avsb",
                                   name=f"avsb_{b}_{qlo}")
                for hl in range(2):
                    nc.vector.tensor_copy(out=av_sb[:, hl, 0:qw],
                                          in_=avs[hl][:, 0:qw])
                # gather the denominator row across 16 partitions so the
                # reciprocal runs on 16 DVE lanes (a single-partition
                # reciprocal is ~16x slower)
                den = spool.tile([16, 64], F32, tag="den", name=f"den_{b}_{qlo}")
                for hl in range(2):
                    nc.sync.dma_start(out=den[8 * hl:8 * hl + 8, 0:qv],
                                      in_=av_sb[64:65, hl, 0:qw])
                rcp = spool.tile([16, 64], F32, tag="rcp", name=f"rcp_{b}_{qlo}")
                nc.vector.reciprocal(rcp[:, 0:qv], den[:, 0:qv])
                db = dpool.tile([2, 512], F32, name=f"db_{b}_{qlo}")
                nc.sync.dma_start(out=db[:, 0:qw], in_=rcp[:, 0:qv])
                db_ap = db[:]
                # partition-broadcast of the reciprocals:
                # rb[d, hl, q] = db[hl, q]  (keeps both TT reads at base
                # partition 0 — TensorTensor requires equal input bases)
                rb = spool.tile([64, 2, 512], F32, tag="rbc",
                                name=f"rb_{b}_{qlo}")
                nc.sync.dma_start(
                    out=rb[:, :, 0:qw],
                    in_=bass.AP(tensor=db_ap.tensor, offset=db_ap.offset,
                                ap=[[0, 64], [512, 2], [1, qw]]))
                for hl in range(2):
                    nc.vector.tensor_mul(
                        projin[hl * 64:(hl + 1) * 64, b, qlo:qlo + qw],
                        av_sb[0:64, hl, 0:qw], rb[:, hl, 0:qw])

            def attn_norm(b, qlo, qw):
                norm_block(b, qlo, qw, attn_block(b, qlo, qw))

            pps = {}

            def proj_partial(nt, ring=None):
                # accumulate the b2=0..2 contributions; these depend only on
                # batches 0-2 being normalized, so they fill the PE while the
                # b3 attention block that gates the wave is still running
                if ring == "sT":
                    t = ps_sT.tile([128, 2, 512], F32, tag="sT",
                                   name=f"ppsT_{nt}")
                    p0, p1 = t[:, 0, :], t[:, 1, :]
                else:
                    p0 = ps_qk.tile([128, 512], F32, tag="qk",
                                    name=f"pps0_{nt}")
                    p1 = ps_qk.tile([128, 512], F32, tag="qk",
                                    name=f"pps1_{nt}")
                pps[nt] = (p0, p1)
                for b2 in range(B - 1):
                    nc.tensor.matmul(
                        p0[:], projin[:, b2, nt * 128:(nt + 1) * 128],
                        wp[:, b2, 0:512], start=(b2 == 0), stop=False)
                    nc.tensor.matmul(
                        p1[:], projin[:, b2, nt * 128:(nt + 1) * 128],
                        wp[:, b2, 512:1024], start=(b2 == 0), stop=False)

            def proj_final(nt):
                p0, p1 = pps[nt]
                nc.tensor.matmul(
                    p0[:], projin[:, 3, nt * 128:(nt + 1) * 128],
                    wp[:, 3, 0:512], start=False, stop=True)
                nc.tensor.matmul(
                    p1[:], projin[:, 3, nt * 128:(nt + 1) * 128],
                    wp[:, 3, 512:1024], start=False, stop=True)
                for dt, p in ((0, p0), (1, p1)):
                    ocp = opool.tile([128, 512], BF, tag="o",
                                     name=f"ocp_{nt}_{dt}")
                    nc.vector.tensor_copy(out=ocp[:], in_=p[:])
                    nc.sync.dma_start(
                        out=out_e[nt * 128:(nt + 1) * 128,
                                  dt * 512:(dt + 1) * 512],
                        in_=ocp[:])

            # schedule: qkv emitted one-batch-ahead of the attention block
            # that needs it, but as LATE as dependencies allow — the tile
            # scheduler treats emission order as priority, so late-emitted
            # qkv matmuls become the filler that keeps the PE dense through
            # the exp-paced attention pipeline.  The endgame emits proj
            # waves as soon as their token range is normalized, with
            # (b3,qt1) split in halves to unlock the last waves earlier.
            qkv_qk(0)
            if V2_WARMUP:      # gap filler while x chunks stream in
                for i in range(16):
                    nc.tensor.matmul(wps[:, 0, 0:128], wqk[:, 0, 0:128],
                                     wqk[:, 0, 0:128], start=True, stop=True)
            qkv_v(0)
            qkv_qk(1)
            qkv_v(1)
            attn_norm(0, 0, 512)
            attn_norm(0, 512, 512)
            qkv_qk(2)
            qkv_v(2)
            attn_norm(1, 0, 512)
            qkv_qk(3)
            attn_norm(1, 512, 512)
            qkv_v(3)
            attn_norm(2, 0, 512)
            attn_norm(2, 512, 512)
            proj_partial(0)
            attn_norm(3, 0, 512)
            proj_final(0)
            for nt in (1, 2, 3):
                proj_partial(nt)
                proj_final(nt)
            attn_norm(3, 512, 256)
            for nt in (4, 5):
                proj_partial(nt)
                proj_final(nt)
            attn_norm(3, 768, 256)
            proj_partial(6, ring="sT")
            proj_partial(7, ring="sT")
            proj_final(6)
            proj_final(7)

    nc.compile()
    return nc


def _prep_core(i, xT, w_qkv, w_proj):
    """Per-core input shards (host-side layout absorption)."""
    h0 = 2 * i
    rows = np.concatenate([np.arange(h0 * HD, (h0 + 1) * HD),
                           np.arange((h0 + 1) * HD, (h0 + 2) * HD)])
    w_qk = np.concatenate([w_qkv[rows] * 0.125, w_qkv[C + rows]], axis=0).T
    w_v = w_qkv[2 * C + rows].T
    hh = np.array([h0, h0 + 1])
    cg = ((hh % 4)[None, :, None] * 256
          + np.arange(B)[:, None, None] * 64
          + np.arange(HD)[None, None, :])          # [b2, hl, d]
    w_p = w_proj[:, cg.reshape(-1)].T              # [512, 1024]
    return {
        "xT": xT,
        "w_qk": np.ascontiguousarray(w_qk, dtype=bf16),
        "w_v": np.ascontiguousarray(w_v, dtype=bf16),
        "w_p": np.ascontiguousarray(w_p, dtype=bf16),
    }


def _run(inputs, trace=False, **kw):
    x = np.asarray(inputs["x"], dtype=np.float32)
    w_qkv = np.asarray(inputs["w_qkv"], dtype=np.float32)
    w_proj = np.asarray(inputs["w_proj"], dtype=np.float32)
    b_proj = np.asarray(inputs["b_proj"], dtype=np.float32)

    if "nc" not in _NC_CACHE:
        _NC_CACHE["nc"] = build_nc()
    nc = _NC_CACHE["nc"]

    xT = np.ascontiguousarray(
        x.transpose(2, 1, 0).reshape(C, NT), dtype=bf16)
    in_maps = [_prep_core(i, xT, w_qkv, w_proj) for i in range(NCORES)]
    res = run_bass_kernel_spmd(nc, in_maps, core_ids=list(range(NCORES)),
                               trace=trace, **kw)
    out = np.empty((N, B, C), np.float32)
    for j in range(4):
        out[:, j, :] = (res.results[2 * j]["out"].astype(np.float32)
                        + res.results[2 * j + 1]["out"].astype(np.float32)
                        + b_proj)
    return out, res


def kernel(**inputs) -> np.ndarray:
    out, _ = _run(inputs, trace=False)
    return out


# revision 37
# speedup vs baseline: 1.1118x; 1.1118x over previous
"""Trainium2 8-core kernel for nn_Attention_88948772700322.

Reference computes (N=1024, B=4, C=1024, H=16, hd=64):
    qkv = x @ w_qkv.T                      [N,B,3C]
    q,k,v per (b,h); attn = softmax(q k^T / 8) v
    out = (attn.transpose(2,1,0,3)).reshape(N,B,C) @ w_proj.T + b_proj
The reshape interleaves H and B: proj-input channel c of output-batch bn is
attention head h = 4*bn + c//256, original batch b2 = (c%256)//64, dim d = c%64.

Sharding: tensor-parallel over heads — core i owns heads {2i, 2i+1}, all
batches/tokens (6.44 GFLOP/core, perfectly balanced).  Each core computes a
partial projection over its 512 proj-input channels for output batch bn=i//2;
host sums core pairs (the "all-reduce after proj" realized in unshard).

Host-side prep absorbs every layout nuisance:
  - xT [C, B*N] bf16, tokens batch-major  -> qkv needs no on-chip transpose
  - w_qk [C, 256] (cols q_h0,q_h1,k_h0,k_h1), q pre-scaled by 1/8
  - w_v  [C, 128] (cols v_h0,v_h1)
  - w_p  [512, 1024] = w_proj columns permuted to (b2, h_local, d) row order
On-chip per core: qk^T via PE (d-major), v via PE (token-major), scores
computed transposed (keys on partitions), softmax without max-subtraction
(scores are O(1) by construction), denominator via ones-column in V,
normalization by DMA-partition-broadcast reciprocal, partial proj n-major.

Schedule notes (v2):
  - input DMAs issued chunk-interleaved (wqk[kc] with xc[0][kc]) so the first
    qk matmuls start ~1us in; wp loads last.
  - a few dummy matmuls + one dummy exp at t0 warm the PE HAM clock-gate and
    preload the ACT exp table during the initial DMA window.
  - all PSUM pools stay open for the whole kernel (no mid-kernel pool-close
    DRAIN); proj accumulators reuse the qkv psum ring.
  - the last attention block (b3,qt1) is split in token halves and the eight
    proj waves are emitted as soon as their token range has been normalized,
    so the tile scheduler can interleave proj matmuls into the exp-bound
    endgame; output is written bf16 (host sums partials in f32).
"""

import numpy as np
import ml_dtypes

import concourse.bass as bass
import concourse.mybir as mybir
from concourse import bacc
from concourse.tile import TileContext
from concourse.bass_utils import run_bass_kernel_spmd


N, B, C, H, HD = 1024, 4, 1024, 16, 64
NT = B * N          # 4096 tokens
NCORES = 8
BF = mybir.dt.bfloat16
F32 = mybir.dt.float32
bf16 = ml_dtypes.bfloat16

_NC_CACHE = {}


import os
V2_WARMUP = os.environ.get("V2_WARMUP", "1") == "1"
V2_RB3D = os.environ.get("V2_RB3D", "1") == "1"
V2_SPLIT = os.environ.get("V2_SPLIT", "1") == "1"


def build_nc():
    nc = bacc.Bacc()
    xT_e = nc.declare_dram_parameter("xT", [C, NT], BF, isOutput=False)
    wqk_e = nc.declare_dram_parameter("w_qk", [C, 256], BF, isOutput=False)
    wv_e = nc.declare_dram_parameter("w_v", [C, 128], BF, isOutput=False)
    wp_e = nc.declare_dram_parameter("w_p", [512, C], BF, isOutput=False)
    out_e = nc.declare_dram_parameter("out", [N, C], BF, isOutput=True)

    xT_ap = xT_e[:].rearrange("(co p) t -> p co t", p=128)    # [128, 8, 4096]
    wqk_ap = wqk_e[:].rearrange("(co p) m -> p co m", p=128)  # [128, 8, 256]
    wv_ap = wv_e[:].rearrange("(co p) m -> p co m", p=128)    # [128, 8, 128]
    wp_ap = wp_e[:].rearrange("(b2 p) d -> p b2 d", p=128)    # [128, 4, 1024]

    from contextlib import ExitStack
    with TileContext(nc) as tc:
        with ExitStack() as stk:
            cpool = stk.enter_context(tc.tile_pool(name="const", bufs=1))
            epool = stk.enter_context(tc.tile_pool(name="exp", bufs=6))
            spool = stk.enter_context(tc.tile_pool(name="small", bufs=6))
            opool = stk.enter_context(tc.tile_pool(name="outcp", bufs=8))
            dpool = stk.enter_context(
                tc.tile_pool(name="dram", bufs=4, space="DRAM"))
            ps_qk = stk.enter_context(
                tc.tile_pool(name="ps_qk", bufs=2, space="PSUM"))
            ps_sT = stk.enter_context(
                tc.tile_pool(name="ps_sT", bufs=2, space="PSUM"))
            ps_av = stk.enter_context(
                tc.tile_pool(name="ps_av", bufs=2, space="PSUM"))
            # ---- persistent SBUF tensors -------------------------------
            xc = [[cpool.tile([128, N], BF, name=f"xc_{b}_{kc}")
                   for kc in range(8)] for b in range(B)]
            wqk = cpool.tile([128, 8, 256], BF)
            wv = cpool.tile([128, 8, 128], BF)
            wp = cpool.tile([128, 4, C], BF)
            q_sb = cpool.tile([128, NT], BF)       # [ (h0|h1) d, token ]
            k_sb = cpool.tile([128, NT], BF)
            # v token-major with ones cols: [t_in, t_out, (h0 d64, 1, h1 d64, 1)]
            # cols: 0:64 h0-dims, 64 ones, 65:129 h1-dims, 129 ones
            v_sb = cpool.tile([128, 32, 130], BF)
            projin = cpool.tile([128, B, N], BF)   # [(hl,d), b2, n]

            # ---- warmup: dummy matmuls on the first wqk chunk trip the
            # HAM clock-gate to full speed during the input-DMA window and
            # preload the ACT exp table; they gate only on the very first
            # (64KB) weight DMA. --------------------------------------------
            # (lives in the sT ring: the qk ring is needed immediately by
            # the first qkv block, while the first sT tile isn't touched
            # until attention starts ~25us in)
            wps = None
            if V2_WARMUP:
                wps = ps_sT.tile([128, 2, 512], F32, tag="sT", name="wps")
                for i in range(40):
                    nc.tensor.matmul(wps[:, 0, 0:128], wqk[:, 0, 0:128],
                                     wqk[:, 0, 0:128], start=True, stop=True)
                ewarm = spool.tile([128, 128], BF, tag="ewarm", name="ewarm")
                nc.scalar.activation(ewarm[:], wps[:, 0, 0:128],
                                     mybir.ActivationFunctionType.Exp)

            # ---- input DMAs (single sync queue, in consumption order;
            # parallel queues measurably contend and lose) ---------------
            for kc in range(8):
                nc.sync.dma_start(out=wqk[:, kc, :], in_=wqk_ap[:, kc, :])
                nc.sync.dma_start(out=xc[0][kc][:], in_=xT_ap[:, kc, 0:N])
            nc.sync.dma_start(out=wv[:], in_=wv_ap)
            for b in range(1, B):
                for kc in range(8):
                    nc.sync.dma_start(out=xc[b][kc][:],
                                      in_=xT_ap[:, kc, b * N:(b + 1) * N])
            nc.sync.dma_start(out=wp[:], in_=wp_ap)

            nc.vector.memset(v_sb[:, :, 64:65], 1.0)
            nc.vector.memset(v_sb[:, :, 129:130], 1.0)

            def qkv_qk(b):
                for tc_i in (2 * b, 2 * b + 1):
                    qps = ps_qk.tile([128, 512], F32, tag="qk",
                                     name=f"qps_{b}_{tc_i}")
                    kps = ps_qk.tile([128, 512], F32, tag="qk",
                                     name=f"kps_{b}_{tc_i}")
                    for kc in range(8):
                        j = tc_i - 2 * b
                        nc.tensor.matmul(qps[:], wqk[:, kc, 0:128],
                                         xc[b][kc][:, j * 512:(j + 1) * 512],
                                         start=(kc == 0), stop=(kc == 7))
                        nc.tensor.matmul(kps[:], wqk[:, kc, 128:256],
                                         xc[b][kc][:, j * 512:(j + 1) * 512],
                                         start=(kc == 0), stop=(kc == 7))
                    nc.vector.tensor_copy(
                        out=q_sb[:, tc_i * 512:(tc_i + 1) * 512], in_=qps[:])
                    nc.vector.tensor_copy(
                        out=k_sb[:, tc_i * 512:(tc_i + 1) * 512], in_=kps[:])

            def qkv_v(b):
                for tt in range(8 * b, 8 * b + 8):
                    vps = ps_qk.tile([128, 128], F32, tag="qk", name=f"vps_{tt}")
                    for kc in range(8):
                        nc.tensor.matmul(vps[:],
                                         xc[b][kc][:, (tt - 8 * b) * 128:
                                                    (tt - 8 * b + 1) * 128],
                                         wv[:, kc, :],
                                         start=(kc == 0), stop=(kc == 7))
                    nc.vector.tensor_copy(out=v_sb[:, tt, 0:64],
                                          in_=vps[:, 0:64])
                    nc.vector.tensor_copy(out=v_sb[:, tt, 65:129],
                                          in_=vps[:, 64:128])

            def attn_block(b, qlo, qw):
                """Attention for q-tokens [qlo, qlo+qw) of batch b."""
                q_sl = slice(b * N + qlo, b * N + qlo + qw)
                av0 = ps_av.tile([65, 512], F32, tag="av",
                                 name=f"av0_{b}_{qlo}")
                av1 = ps_av.tile([65, 512], F32, tag="av",
                                 name=f"av1_{b}_{qlo}")
                avs = [av0, av1]
                for kc in range(8):
                    k_sl = slice(b * N + kc * 128, b * N + (kc + 1) * 128)
                    # [128, 2, 512]: each head's half in its own PSUM bank,
                    # even for qw<512 (two concurrent tile_position matmuls
                    # must not write the same bank)
                    sT = ps_sT.tile([128, 2, 512], F32, tag="sT",
                                    name=f"sT_{b}_{qlo}_{kc}")
                    for hl in range(2):
                        nc.tensor.matmul(
                            sT[:, hl, 0:qw],
                            k_sb[hl * 64:(hl + 1) * 64, k_sl],
                            q_sb[hl * 64:(hl + 1) * 64, q_sl],
                            start=True, stop=True,
                            tile_position=(hl * 64, 0))
                    e = epool.tile([128, 2, 512], BF, tag="e",
                                   name=f"e_{b}_{qlo}_{kc}")
                    nc.scalar.activation(
                        e[:, :, 0:qw], sT[:, :, 0:qw],
                        mybir.ActivationFunctionType.Exp)
                    for hl in range(2):
                        nc.tensor.matmul(
                            avs[hl][:, 0:qw],
                            v_sb[:, 8 * b + kc, hl * 65:(hl + 1) * 65],
                            e[:, hl, 0:qw],
                            start=(kc == 0), stop=(kc == 7))
                return avs

            def norm_block(b, qlo, qw, avs):
                # evacuate av psum -> sbuf (frees psum; DMA can then read den)
                qv = qw // 8
                av_sb = spool.tile([65, 2, 512], F32, tag="avsb",
                                   name=f"avsb_{b}_{qlo}")
                for hl in range(2):
                    nc.vector.tensor_copy(out=av_sb[:, hl, 0:qw],
                                          in_=avs[hl][:, 0:qw])
                # gather the denominator row across 16 partitions so the
                # reciprocal runs on 16 DVE lanes (a single-partition
                # reciprocal is ~16x slower)
                den = spool.tile([16, 64], F32, tag="den", name=f"den_{b}_{qlo}")
                for hl in range(2):
                    nc.gpsimd.dma_start(out=den[8 * hl:8 * hl + 8, 0:qv],
                                        in_=av_sb[64:65, hl, 0:qw])
                rcp = spool.tile([16, 64], F32, tag="rcp", name=f"rcp_{b}_{qlo}")
                nc.vector.reciprocal(rcp[:, 0:qv], den[:, 0:qv])
                db = dpool.tile([2, 512], F32, name=f"db_{b}_{qlo}")
                nc.gpsimd.dma_start(out=db[:, 0:qw], in_=rcp[:, 0:qv])
                db_ap = db[:]
                # partition-broadcast of the reciprocals:
                # rb[d, hl, q] = db[hl, q]  (keeps both TT reads at base
                # partition 0 — TensorTensor requires equal input bases)
                rb = spool.tile([64, 2, 512], F32, tag="rbc",
                                name=f"rb_{b}_{qlo}")
                nc.gpsimd.dma_start(
                    out=rb[:, :, 0:qw],
                    in_=bass.AP(tensor=db_ap.tensor, offset=db_ap.offset,
                                ap=[[0, 64], [512, 2], [1, qw]]))
                for hl in range(2):
                    nc.vector.tensor_mul(
                        projin[hl * 64:(hl + 1) * 64, b, qlo:qlo + qw],
                        av_sb[0:64, hl, 0:qw], rb[:, hl, 0:qw])

            def attn_norm(b, qlo, qw):
                norm_block(b, qlo, qw, attn_block(b, qlo, qw))

            pps = {}

            def proj_partial(nt, ring=None):
                # accumulate the b2=0..2 contributions; these depend only on
                # batches 0-2 being normalized, so they fill the PE while the
                # b3 attention block that gates the wave is still running
                if ring == "sT":
                    t = ps_sT.tile([128, 2, 512], F32, tag="sT",
                                   name=f"ppsT_{nt}")
                    p0, p1 = t[:, 0, :], t[:, 1, :]
                else:
                    p0 = ps_qk.tile([128, 512], F32, tag="qk",
                                    name=f"pps0_{nt}")
                    p1 = ps_qk.tile([128, 512], F32, tag="qk",
                                    name=f"pps1_{nt}")
                pps[nt] = (p0, p1)
                for b2 in range(B - 1):
                    nc.tensor.matmul(
                        p0[:], projin[:, b2, nt * 128:(nt + 1) * 128],
                        wp[:, b2, 0:512], start=(b2 == 0), stop=False)
                    nc.tensor.matmul(
                        p1[:], projin[:, b2, nt * 128:(nt + 1) * 128],
                        wp[:, b2, 512:1024], start=(b2 == 0), stop=False)

            def proj_final(nt):
                p0, p1 = pps[nt]
                nc.tensor.matmul(
                    p0[:], projin[:, 3, nt * 128:(nt + 1) * 128],
                    wp[:, 3, 0:512], start=False, stop=True)
                nc.tensor.matmul(
                    p1[:], projin[:, 3, nt * 128:(nt + 1) * 128],
                    wp[:, 3, 512:1024], start=False, stop=True)
                for dt, p in ((0, p0), (1, p1)):
                    ocp = opool.tile([128, 512], BF, tag="o",
                                     name=f"ocp_{nt}_{dt}")
                    nc.vector.tensor_copy(out=ocp[:], in_=p[:])
                    nc.sync.dma_start(
                        out=out_e[nt * 128:(nt + 1) * 128,
                                  dt * 512:(dt + 1) * 512],
                        in_=ocp[:])

            # schedule: qkv emitted one-batch-ahead of the attention block
            # that needs it, but as LATE as dependencies allow — the tile
            # scheduler treats emission order as priority, so late-emitted
            # qkv matmuls become the filler that keeps the PE dense through
            # the exp-paced attention pipeline.  The endgame emits proj
            # waves as soon as their token range is normalized, with
            # (b3,qt1) split in halves to unlock the last waves earlier.
            qkv_qk(0)
            if V2_WARMUP:      # gap filler while x chunks stream in
                for i in range(16):
                    nc.tensor.matmul(wps[:, 0, 0:128], wqk[:, 0, 0:128],
                                     wqk[:, 0, 0:128], start=True, stop=True)
            qkv_v(0)
            qkv_qk(1)
            qkv_v(1)
            attn_norm(0, 0, 512)
            attn_norm(0, 512, 512)
            qkv_qk(2)
            qkv_v(2)
            attn_norm(1, 0, 512)
            qkv_qk(3)
            attn_norm(1, 512, 512)
            qkv_v(3)
            attn_norm(2, 0, 512)
            attn_norm(2, 512, 512)
            proj_partial(0)
            attn_norm(3, 0, 512)
            proj_final(0)
            for nt in (1, 2, 3):
                proj_partial(nt)
                proj_final(nt)
            attn_norm(3, 512, 256)
            for nt in (4, 5):
                proj_partial(nt)
                proj_final(nt)
            attn_norm(3, 768, 256)
            proj_partial(6, ring="sT")
            proj_partial(7, ring="sT")
            proj_final(6)
            proj_final(7)

    nc.compile()
    return nc


def _prep_core(i, xT, w_qkv, w_proj):
    """Per-core input shards (host-side layout absorption)."""
    h0 = 2 * i
    rows = np.concatenate([np.arange(h0 * HD, (h0 + 1) * HD),
                           np.arange((h0 + 1) * HD, (h0 + 2) * HD)])
    w_qk = np.concatenate([w_qkv[rows] * 0.125, w_qkv[C + rows]], axis=0).T
    w_v = w_qkv[2 * C + rows].T
    hh = np.array([h0, h0 + 1])
    cg = ((hh % 4)[None, :, None] * 256
          + np.arange(B)[:, None, None] * 64
          + np.arange(HD)[None, None, :])          # [b2, hl, d]
    w_p = w_proj[:, cg.reshape(-1)].T              # [512, 1024]
    return {
        "xT": xT,
        "w_qk": np.ascontiguousarray(w_qk, dtype=bf16),
        "w_v": np.ascontiguousarray(w_v, dtype=bf16),
        "w_p": np.ascontiguousarray(w_p, dtype=bf16),
    }


def _run(inputs, trace=False, **kw):
    x = np.asarray(inputs["x"], dtype=np.float32)
    w_qkv = np.asarray(inputs["w_qkv"], dtype=np.float32)
    w_proj = np.asarray(inputs["w_proj"], dtype=np.float32)
    b_proj = np.asarray(inputs["b_proj"], dtype=np.float32)

    if "nc" not in _NC_CACHE:
        _NC_CACHE["nc"] = build_nc()
    nc = _NC_CACHE["nc"]

    xT = np.ascontiguousarray(
        x.transpose(2, 1, 0).reshape(C, NT), dtype=bf16)
    in_maps = [_prep_core(i, xT, w_qkv, w_proj) for i in range(NCORES)]
    res = run_bass_kernel_spmd(nc, in_maps, core_ids=list(range(NCORES)),
                               trace=trace, **kw)
    out = np.empty((N, B, C), np.float32)
    for j in range(4):
        out[:, j, :] = (res.results[2 * j]["out"].astype(np.float32)
                        + res.results[2 * j + 1]["out"].astype(np.float32)
                        + b_proj)
    return out, res


def kernel(**inputs) -> np.ndarray:
    out, _ = _run(inputs, trace=False)
    return out
